# revision 9
# baseline (speedup 1.0000x reference)
"""Multi-head causal attention on 8 Trainium2 NeuronCores.

Problem: x[4,2048,1024] @ {W_q,W_k,W_v}, 16 heads x d_k=64, causal softmax,
context @ W_o. Sharding: 8 cores = 4 batches x 2 head-groups (tensor
parallel over heads, data parallel over batch). Each core computes, for its
batch b and its 8 heads: projections, causal attention, and a partial
output  context_g @ W_o[g-rows]  [2048,1024]. Host sums the two partials
per batch (the W_o row-split reduction) and stacks batches.

Layout strategy (everything contraction-major; single x transpose):
  xT[D,S]   via PE-transpose of x
  QT[dd,S] = Wq_g.T x.T   (lhsT=Wq chunks, rhs=xT)      f32r
  KT[dd,S], V[S,dd] likewise; V augmented with a ones column per head so
      the context matmul's row 64 yields the softmax denominator l free
  ST[k,q] -> PSUM pairs [128k, 2, 512q];  E = exp(ST/8) one ACT op per
      pair; causal mask via gpsimd affine_select on diagonal halves;
      matmul/exp free dims trimmed to the causal range (floor 256)
  ctxT[65,q] accumulated over k-blocks (lhsT=V_aug, rhs=E halves)
  1/l via partition-spread DVE reciprocal + gpsimd partition_broadcast,
      DVE multiply; ctxT to per-chunk DRAM scratch
  out[q,1024] accumulated over 4 ctx chunks (lhsT=ctxT chunk, rhs=Wo_g)

Schedule: attention is exp(ACT)-throughput-paced, so projection work for
sequence-quarter q+1 is interleaved between attention groups of query-tile
q to keep the PE array busy (idle PE triggers the HAM clock-gate to half
rate, which doubles matmul time for the whole phase). The projection pools
are freed after the qt=2 phase so W_o and the ctx lhsT can be prefetched
into SBUF during the qt=3 phase, removing the DRAM-roundtrip stall before
the output projection.
"""
from contextlib import ExitStack

import numpy as np

import concourse.bacc as bacc
import concourse.mybir as mybir
import concourse.tile as tile
from concourse.bass_utils import run_bass_kernel_spmd
from concourse.masks import make_identity

P = 128
S = 2048
D = 1024
GW = 512          # per-core head-group width (8 heads x 64)
DK = 64
HG = 8
NDC = D // P
NQT = S // 512
NSB = S // P
NCH = GW // P

F32 = mybir.dt.float32
F32R = mybir.dt.float32r
RDT = F32R
SCALE = 0.125
N_CORES = 8


def build():
    nc = bacc.Bacc("TRN2", target_bir_lowering=False, debug=False)
    xb = nc.dram_tensor("xb", [S, D], F32, kind="ExternalInput")
    wq = nc.dram_tensor("wq", [D, GW], F32, kind="ExternalInput")
    wk = nc.dram_tensor("wk", [D, GW], F32, kind="ExternalInput")
    wv = nc.dram_tensor("wv", [D, GW], F32, kind="ExternalInput")
    wo = nc.dram_tensor("wo", [GW, D], F32, kind="ExternalInput")
    outp = nc.dram_tensor("outp", [S, D], F32, kind="ExternalOutput")

    def r(ap):
        return ap.bitcast(RDT) if RDT is F32R else ap

    with tile.TileContext(nc) as tc, \
         tc.tile_pool(name="const", bufs=1) as cpool, \
         tc.tile_pool(name="dram", bufs=1, space="DRAM") as dpool, \
         tc.tile_pool(name="stores", bufs=1) as stores, \
         tc.tile_pool(name="qtp", bufs=2) as qtp, \
         tc.tile_pool(name="e", bufs=6) as epool, \
         tc.tile_pool(name="lwork", bufs=1) as lwork, \
         tc.tile_pool(name="cstage", bufs=2) as cstage, \
         tc.tile_pool(name="ps_sc", bufs=2, space="PSUM") as ps_sc, \
         tc.tile_pool(name="ps_cx", bufs=2, space="PSUM") as ps_cx, \
         tc.tile_pool(name="ps_pj", bufs=2, space="PSUM") as ps_pj:

        proj_stack = ExitStack()
        wpool = proj_stack.enter_context(tc.tile_pool(name="wqkv", bufs=1))
        xin = proj_stack.enter_context(tc.tile_pool(name="xin", bufs=2))
        xtp = proj_stack.enter_context(tc.tile_pool(name="xt", bufs=1))

        ident = cpool.tile([P, P], F32, tag="ident")
        make_identity(nc, ident[:])

        kT = stores.tile([P, NCH, S], RDT, tag="kT")
        v_aug = stores.tile([P, NSB, HG, DK + 1], RDT, tag="v")
        nc.vector.tensor_copy(
            v_aug[:, :, :, DK:DK + 1],
            nc.const_aps.tensor(1.0, (P, NSB, HG, 1), F32))
        ctx_dram = [dpool.tile([P, S], F32, name=f"ctxd{c}", tag=f"ctxd{c}")
                    for c in range(NCH)]
        qT_tiles = {}
        xt_cur = {}
        ctxl = {}

        # ---- projection emission units for one sequence-quarter ----------
        def proj_units(q4):
            units = []

            def load_w():
                wq_t = wpool.tile([P, NDC, GW], RDT, tag="wq")
                wk_t = wpool.tile([P, NDC, GW], RDT, tag="wk")
                wv_t = wpool.tile([P, NDC, GW], RDT, tag="wv")
                nc.sync.dma_start(wk_t[:], r(wk.rearrange("(dc p) n -> p dc n", p=P)))
                nc.sync.dma_start(wq_t[:], r(wq.rearrange("(dc p) n -> p dc n", p=P)))
                nc.sync.dma_start(wv_t[:], r(wv.rearrange("(dc p) n -> p dc n", p=P)))
                proj_units.w = (wq_t, wk_t, wv_t)

            def start():
                xt_cur[0] = xtp.tile([P, NDC, 512], RDT, tag="xt", name=f"xt{q4}")
                qT_tiles[q4] = qtp.tile([P, NCH, 512], RDT, tag="qT", name=f"qT{q4}")
            units.append(start)

            def transpose_block(sbl):
                xt_q = xt_cur[0]
                sb = q4 * 4 + sbl
                x_blk = xin.tile([P, D], F32, tag="xin")
                nc.sync.dma_start(x_blk[:], xb[sb * P:(sb + 1) * P, :])
                for g in range(2):
                    tp_ps = ps_pj.tile([P, 4, P], F32, tag="pj")
                    for i in range(4):
                        dc = g * 4 + i
                        nc.tensor.transpose(
                            tp_ps[:, i, :],
                            x_blk[:, dc * P:(dc + 1) * P], ident[:])
                    nc.vector.tensor_copy(
                        xt_q[:, g * 4:(g + 1) * 4, sbl * P:(sbl + 1) * P],
                        tp_ps[:].bitcast(F32))
            for sbl in range(4):
                units.append(lambda sbl=sbl: transpose_block(sbl))
                if q4 == 0 and sbl == 0:
                    units.append(load_w)

            def qk_proj(w_i, j):
                w_t = proj_units.w[w_i]
                dst = qT_tiles[q4] if w_i == 0 else kT
                pj = ps_pj.tile([P, 512], F32, tag="pj")
                for dc in range(NDC):
                    nc.tensor.matmul(pj[:], w_t[:, dc, j * P:(j + 1) * P],
                                     xt_cur[0][:, dc, :],
                                     start=(dc == 0), stop=(dc == NDC - 1))
                if w_i == 0:
                    nc.vector.tensor_copy(dst[:, j, :], pj[:].bitcast(F32))
                else:
                    nc.vector.tensor_copy(
                        dst[:, j, q4 * 512:(q4 + 1) * 512], pj[:].bitcast(F32))

            def v_proj(sbl):
                sb = q4 * 4 + sbl
                pj = ps_pj.tile([P, 512], F32, tag="pj")
                for dc in range(NDC):
                    nc.tensor.matmul(pj[:], xt_cur[0][:, dc, sbl * P:(sbl + 1) * P],
                                     proj_units.w[2][:, dc, :],
                                     start=(dc == 0), stop=(dc == NDC - 1))
                nc.vector.tensor_copy(v_aug[:, sb, :, :DK], pj[:].bitcast(F32))

            for j in range(NCH):
                units.append(lambda j=j: qk_proj(1, j))   # K first
            for j in range(NCH):
                units.append(lambda j=j: qk_proj(0, j))   # then Q
            for sbl in range(4):
                units.append(lambda sbl=sbl: v_proj(sbl))
            return units

        # ---- attention group emitters ------------------------------------
        def vstart(kb, qt):
            # first causally-valid q in the tile for k-block kb, capped so
            # trimmed matmul free dims stay >= 256 (f32r fast regime)
            return min(max(0, P * (kb - 4 * qt)), 256)

        def emit_scores(h, qt):
            po = 64 * (h % 2)
            j = h // 2
            q_ap = qT_tiles[qt][po:po + 64, j, :]
            kt_h = kT[po:po + 64, j, :]
            e_pairs = []
            for pr in range(2 * (qt + 1)):
                vs0 = vstart(2 * pr, qt)
                s_ps = ps_sc.tile([P, 2, 512], F32, tag="sc")
                for i in range(2):
                    kb = 2 * pr + i
                    vs = vstart(kb, qt)
                    nc.tensor.matmul(s_ps[:, i, vs:],
                                     kt_h[:, kb * P:(kb + 1) * P],
                                     q_ap[:, vs:], start=True, stop=True)
                e_sb = epool.tile([P, 2, 512], RDT, tag="e")
                nc.scalar.activation(e_sb[:, :, vs0:], s_ps[:, :, vs0:],
                                     mybir.ActivationFunctionType.Exp,
                                     scale=SCALE)
                for i in range(2):
                    kb = 2 * pr + i
                    if kb >= 4 * qt:
                        # zero the below-diagonal part and stale-exp overhang
                        nc.gpsimd.affine_select(
                            out=e_sb[:, i, vs0:], in_=e_sb[:, i, vs0:],
                            compare_op=mybir.AluOpType.is_ge,
                            fill=0.0, base=512 * qt - kb * P + vs0,
                            pattern=[[1, 512 - vs0]], channel_multiplier=-1)
                e_pairs.append(e_sb)
            return e_pairs

        def emit_ctx(h, qt, e_pairs):
            po = 64 * (h % 2)
            j = h // 2
            nk = 4 * (qt + 1)
            ctx_ps = ps_cx.tile([P, 512], F32, tag="cx")
            for kb in range(nk):
                vs = vstart(kb, qt)
                nc.tensor.matmul(ctx_ps[0:DK + 1, vs:],
                                 v_aug[:, kb, h, :],
                                 e_pairs[kb // 2][:, kb % 2, vs:],
                                 start=(kb == 0), stop=(kb == nk - 1),
                                 skip_group_check=True)
            # 1/l: spread 512 l values over 8 partitions so the DVE
            # reciprocal (serial within a partition) is cheap, gather back,
            # broadcast over the 64 ctx partitions.
            lrow = lwork.tile([P, 512], F32, tag="lrow")
            nc.vector.tensor_copy(lrow[64:65, :], ctx_ps[64:65, :])
            lsp = lwork.tile([P, 512], F32, tag="lsp")
            nc.sync.dma_start(lsp[0:8, 0:64], lrow[64:65, :])
            nc.vector.reciprocal(lsp[0:8, 64:128], lsp[0:8, 0:64])
            linv = lwork.tile([P, 512], F32, tag="linv")
            nc.sync.dma_start(linv[0:1, :], lsp[0:8, 64:128])
            lrep = lwork.tile([P, 512], F32, tag="lrep")
            nc.gpsimd.partition_broadcast(lrep[0:DK, :], linv[0:1, :],
                                          channels=DK)
            stg = cstage.tile([P, 512], F32, tag="stg")
            nc.vector.tensor_mul(out=stg[0:DK, :], in0=ctx_ps[0:DK, :],
                                 in1=lrep[0:DK, :])
            nc.sync.dma_start(
                ctx_dram[j][po:po + 64, qt * 512:(qt + 1) * 512],
                stg[0:DK, :])
            if qt == NQT - 1 and "t" in ctxl:
                # mirror the freshly written slice into the resident lhsT
                nc.sync.dma_start(
                    ctxl["t"][po:po + 64, j, qt * 512:(qt + 1) * 512],
                    r(ctx_dram[j][po:po + 64, qt * 512:(qt + 1) * 512]))

        # ---- interleaved emission: quarters 0..2 + attention qt 0..2 -----
        for u in proj_units(0):
            u()
        prev = None
        for qt in range(NQT - 1):
            pu = proj_units(qt + 1)
            pi = 0
            for h in range(HG):
                e_pairs = emit_scores(h, qt)
                if prev is not None:
                    emit_ctx(*prev)
                prev = (h, qt, e_pairs)
                take = ((h + 1) * len(pu)) // HG - (h * len(pu)) // HG
                for _ in range(take):
                    pu[pi]()
                    pi += 1
        proj_stack.close()

        # ---- qt=3 attention + resident out-projection inputs -------------
        with tc.tile_pool(name="wo", bufs=1) as wop, \
             tc.tile_pool(name="clhs", bufs=1) as clhs, \
             tc.tile_pool(name="ostage", bufs=3) as ostage:
            emit_ctx(*prev)        # (h7, qt2): last qt<=2 ctx write
            prev = None
            wo_t = wop.tile([P, NCH, D], RDT, tag="wo")
            nc.sync.dma_start(wo_t[:], r(wo.rearrange("(c p) n -> p c n", p=P)))
            ctx_l = clhs.tile([P, NCH, S], RDT, tag="ctxl")
            ctxl["t"] = ctx_l
            for c in range(NCH):   # bulk-prefetch the qt<=2 regions
                nc.sync.dma_start(ctx_l[:, c, 0:1536],
                                  r(ctx_dram[c][:, 0:1536]))
            qt = NQT - 1
            for h in range(HG):
                e_pairs = emit_scores(h, qt)
                if prev is not None:
                    emit_ctx(*prev)
                prev = (h, qt, e_pairs)
            emit_ctx(*prev)

            for qb in range(NSB):
                for nh in range(2):
                    po_ps = ps_cx.tile([P, 512], F32, tag="cx")
                    for c in range(NCH):
                        nc.tensor.matmul(
                            po_ps[:], ctx_l[:, c, qb * P:(qb + 1) * P],
                            wo_t[:, c, nh * 512:(nh + 1) * 512],
                            start=(c == 0), stop=(c == NCH - 1))
                    ost = ostage.tile([P, 512], F32, tag="ost")
                    nc.vector.tensor_copy(ost[:], po_ps[:])
                    nc.sync.dma_start(
                        outp[qb * P:(qb + 1) * P, nh * 512:(nh + 1) * 512],
                        ost[:])
    nc.compile()
    return nc


_NC_CACHE = None


def _get_nc():
    global _NC_CACHE
    if _NC_CACHE is None:
        _NC_CACHE = build()
    return _NC_CACHE


def _run(x, W_q, W_k, W_v, W_o, trace=False, tmpdir=None):
    x = np.ascontiguousarray(x, dtype=np.float32)
    W_q = np.ascontiguousarray(W_q, dtype=np.float32)
    W_k = np.ascontiguousarray(W_k, dtype=np.float32)
    W_v = np.ascontiguousarray(W_v, dtype=np.float32)
    W_o = np.ascontiguousarray(W_o, dtype=np.float32)
    B = x.shape[0]
    in_maps = []
    for c in range(N_CORES):
        b, g = c // 2, c % 2
        in_maps.append({
            "xb": x[b],
            "wq": np.ascontiguousarray(W_q[:, g * GW:(g + 1) * GW]),
            "wk": np.ascontiguousarray(W_k[:, g * GW:(g + 1) * GW]),
            "wv": np.ascontiguousarray(W_v[:, g * GW:(g + 1) * GW]),
            "wo": np.ascontiguousarray(W_o[g * GW:(g + 1) * GW, :]),
        })
    nc = _get_nc()
    res = run_bass_kernel_spmd(nc, in_maps, core_ids=list(range(N_CORES)),
                               trace=trace, tmpdir=tmpdir)
    out = np.empty((B, S, D), np.float32)
    for b in range(B):
        out[b] = res.results[2 * b]["outp"] + res.results[2 * b + 1]["outp"]
    return out, res


def kernel(x, W_q, W_k, W_v, W_o):
    out, _ = _run(x, W_q, W_k, W_v, W_o)
    return out
